# revision 32
# baseline (speedup 1.0000x reference)
"""Trainium2 Bass kernel for GQA attention with RoPE, causal mask, and
attention sinks (nn_Attention_65094524338392).

Sharding: tensor-parallel by heads across 8 NeuronCores. Core c owns query
heads 4c..4c+3 and kv-head c (NREP=4, so kv-head groups stay aligned with
their query heads). Each core computes QKV projections over the full
sequence for its heads, flash-style causal attention, then an AllToAll
redistributes attention outputs from head-sharding to sequence-sharding so
each core computes the output projection for its 256-row sequence slice.

Math note: the sink scaling folds into the softmax normalizer:
    out = (sum_k exp(s_k) v_k) * sigmoid(lse - sink) / sum_k exp(s_k)
        = (sum_k exp(s_k) v_k) / (sum_k exp(s_k) + exp(sink))
so no logs/sigmoids are needed on device, and because |s| <= ~40 no
max-subtraction is needed for exp stability in fp32 accumulation.
"""

import os
import sys

sys.path.insert(0, "/opt/trn_rl_repo")

import ml_dtypes
import numpy as np

import concourse.bass as bass
import concourse.mybir as mybir
import concourse.tile as tile
from concourse import bacc
from concourse.bass_utils import run_bass_kernel_spmd

# Problem shapes
B, S, DIM = 1, 2048, 2048
NH, NKV, HD = 32, 8, 64
NREP = NH // NKV
SCALE = 1.0 / float(np.sqrt(HD))
NCORES = 8
HPC = NH // NCORES            # query heads per core (4)
QKV = HPC * HD + 2 * HD       # fused qkv output dim per core (384)
QW = HPC * HD                 # query width per core (256)
SB = 512                      # seq block (attention q-block)
NSB = S // SB                 # 4
NT = S // 128                 # 16 seq tiles
ND = DIM // 128               # 16 contraction tiles
MYS = S // NCORES             # output rows per core (256)

F32 = mybir.dt.float32
BF16 = mybir.dt.bfloat16

_cache = {}

last_exec_time_ns = None


def _install_ntff_shim():
    """Register the NTFF profile hook so trace=True yields exec_time_ns."""
    import types
    if "antenv.axon_hooks" in sys.modules:
        return
    import antenv
    mod = types.ModuleType("antenv.axon_hooks")
    mod._hook = None
    mod.set_axon_ntff_profile_hook = lambda h: setattr(mod, "_hook", h)
    mod.get_axon_ntff_profile_hook = lambda: mod._hook
    sys.modules["antenv.axon_hooks"] = mod
    antenv.axon_hooks = mod
    from trn_agent_boot.trn_boot import _ntff_profile_via_ctypes
    hook = _ntff_profile_via_ctypes("/opt/axon/libaxon_pjrt.so")
    if hook is not None:
        mod._hook = hook


def _build():
    nc = bacc.Bacc("TRN2", target_bir_lowering=False, debug=False,
                   num_devices=NCORES)

    # Input staging happens in declaration order; QKV needs wqkvT + xT0
    # first, so they stage right after the tiny params. cos/sin/masks are
    # pulled on the gpsimd queue so they don't block the sync queue's xT
    # pulls.
    F16 = mybir.dt.float16
    ident_e = nc.declare_dram_parameter("ident", [128, 128], BF16, isOutput=False)
    qkvb_e = nc.declare_dram_parameter("qkvb", [1, QKV], BF16, isOutput=False)
    mask_e = nc.declare_dram_parameter("masks", [128, 128], BF16, isOutput=False)
    wqkvT_e = nc.declare_dram_parameter("wqkvT", [128, ND * QKV], BF16, isOutput=False)
    # x^T is transposed host-side and staged as one param per 512-row seq
    # block so phase B can start as soon as block 0 lands
    xT_es = [nc.declare_dram_parameter(f"xT{s}", [128, ND * SB], BF16,
                                       isOutput=False) for s in range(2)]
    cosd_e = nc.declare_dram_parameter("cosd", [128, NT * HD], F16, isOutput=False)
    nsin_e = nc.declare_dram_parameter("nsin", [128, NT * HD // 2], F16, isOutput=False)
    psin_e = nc.declare_dram_parameter("psin", [128, NT * HD // 2], F16, isOutput=False)
    sinks_e = nc.declare_dram_parameter("sinks4", [1, HPC], F32, isOutput=False)
    xT_es += [nc.declare_dram_parameter(f"xT{s}", [128, ND * SB], BF16,
                                        isOutput=False) for s in range(2, 3)]
    wob_e = nc.declare_dram_parameter("wob", [1, DIM], BF16, isOutput=False)
    xT_es += [nc.declare_dram_parameter(f"xT{s}", [128, ND * SB], BF16,
                                        isOutput=False) for s in range(3, NSB)]
    woT_e = nc.declare_dram_parameter("woT", [128, ND * DIM], BF16, isOutput=False)
    out_e = nc.declare_dram_parameter("out", [MYS, DIM], F32, isOutput=True)

    with tile.TileContext(nc) as tc:
        with tc.tile_pool(name="const", bufs=1) as cp, \
             tc.tile_pool(name="xT", bufs=2) as xtp, \
             tc.tile_pool(name="rope", bufs=3) as rp, \
             tc.tile_pool(name="qkr", bufs=5) as qkrp, \
             tc.tile_pool(name="pt", bufs=5) as ptp, \
             tc.tile_pool(name="ep", bufs=2) as epp, \
             tc.tile_pool(name="fin", bufs=3) as fnp, \
             tc.tile_pool(name="dram", bufs=1, space="DRAM") as dp:

            # ---- constants ---- sync queue carries only what QKV needs
            # (ident/qkvb/wqkvT, then the xT pulls in phase B); everything
            # else rides the gpsimd queue in parallel.
            ident_sb = cp.tile([128, 128], BF16)
            nc.sync.dma_start(ident_sb[:], ident_e[:])
            qkvb_sb = cp.tile([1, QKV], BF16)
            nc.sync.dma_start(qkvb_sb[:], qkvb_e[:])
            wqkvT_sb = cp.tile([128, ND, QKV], BF16)
            nc.sync.dma_start(wqkvT_sb[:], wqkvT_e[:].rearrange(
                "p (o f) -> p o f", o=ND))
            # gpsimd memsets first (instant, and the warm-up matmuls gate
            # on warm_sb), then the gpsimd const pulls
            ones_sb = cp.tile([1, 128], BF16)
            nc.gpsimd.memset(ones_sb[:], 1.0)
            ones512 = cp.tile([1, 512], BF16)
            nc.gpsimd.memset(ones512[:], 1.0)
            warm_sb = cp.tile([128, 512], BF16)
            nc.gpsimd.memset(warm_sb[:], 0.0)
            v_sb = cp.tile([128, NT, HD + 1], BF16)
            nc.gpsimd.memset(v_sb[:, :, HD:HD + 1], 1.0)
            # zero the pt pool buffers once: diagonal-trimmed exp leaves
            # the masked column range unwritten, and the mask multiply
            # must see finite values there (not uninitialized SBUF)
            for pi in range(5):
                ptz = ptp.tile([128, 1024], BF16, tag="pt", name=f"ptz{pi}")
                nc.gpsimd.memset(ptz[:, :], 0.0)
            mask_sb = cp.tile([128, 128], BF16)
            nc.gpsimd.dma_start(mask_sb[:], mask_e[:])
            cos_sb = cp.tile([128, NT, HD], F16)
            nc.gpsimd.dma_start(cos_sb[:], cosd_e[:].rearrange(
                "p (o f) -> p o f", o=NT))
            nsin_sb = cp.tile([128, NT, HD // 2], F16)
            nc.gpsimd.dma_start(nsin_sb[:], nsin_e[:].rearrange(
                "p (o f) -> p o f", o=NT))
            psin_sb = cp.tile([128, NT, HD // 2], F16)
            nc.gpsimd.dma_start(psin_sb[:], psin_e[:].rearrange(
                "p (o f) -> p o f", o=NT))
            sinks_sb = cp.tile([1, HPC], F32)
            nc.gpsimd.dma_start(sinks_sb[:], sinks_e[:])
            es_sb = cp.tile([1, HPC], F32)
            nc.scalar.activation(es_sb[:], sinks_sb[:],
                                 mybir.ActivationFunctionType.Exp)
            wob_sb = cp.tile([1, DIM], BF16)
            nc.gpsimd.dma_start(wob_sb[:], wob_e[:])
            woT_sb = cp.tile([128, ND, DIM], BF16)

            # PE warm-up: a gapless burst of dummy matmuls during the
            # initial input DMAs releases the HAM clock throttle (1.2 ->
            # 2.4 GHz) before real work begins. No data dependencies.
            with tc.tile_pool(name="warm", bufs=1, space="PSUM") as wpp:
                warm_ps = wpp.tile([128, 512], F32, tag="warm")
                for _ in range(30):
                    nc.tensor.matmul(warm_ps[:], warm_sb[:, 0:128],
                                     warm_sb[:], start=True, stop=True)

            # persistent activations: q head-pairs packed in partition halves,
            # k duplicated into both halves (for tile_position row packing)
            qP = [cp.tile([128, S], BF16, name=f"qP{g}") for g in range(HPC // 2)]
            kTd = cp.tile([128, S], BF16)
            oT = [cp.tile([HD, S], BF16, name=f"oT{h}") for h in range(HPC)]

            # one A2A per head pair: the first fires after the g=0 pass and
            # overlaps with the g=1 attention pass
            a2a_in = [dp.tile([S // 2, MYS], BF16, name=f"a2ai{g}")
                      for g in range(HPC // 2)]
            a2a_out = [dp.tile([S // 2, MYS], BF16, name=f"a2ao{g}")
                       for g in range(HPC // 2)]
            # gathered attention outputs: ag_sb[g] block c holds global head
            # rows [c*256 + 128*g, c*256 + 128*(g+1)) = woT block 2c+g
            ag_sb = [cp.tile([128, NCORES, MYS], BF16, name=f"ag{g}")
                     for g in range(HPC // 2)]

            # ---- phase B: QKV projections + rope + transposes ----
            with tc.tile_pool(name="ppB", bufs=2, space="PSUM") as ppB:
                for s in range(NSB):
                    # x^T was transposed host-side; two bulk loads per block
                    xT_s = xtp.tile([128, ND, SB], BF16, tag="xT")
                    xsrc = xT_es[s][:].rearrange("p (d f) -> p d f", d=ND)
                    nc.sync.dma_start(xT_s[:, 0:ND // 2, :],
                                      xsrc[:, 0:ND // 2, :])
                    nc.sync.dma_start(xT_s[:, ND // 2:, :],
                                      xsrc[:, ND // 2:, :])
                    if s == 0:
                        # dependency-gated warm-up: fires as soon as the first
                        # x^T tile lands, so QKV starts at the warm PE clock
                        warm2_ps = ppB.tile([128, 512], F32, tag="warmB",
                                            bufs=1)
                        for _ in range(20):
                            nc.tensor.matmul(warm2_ps[:], xT_s[:, 0, 0:128],
                                             warm_sb[:], start=True, stop=True)
                    qkr_tiles = []
                    for tt in range(4):
                        t = 4 * s + tt

                        # fused qkv projection for this seq tile
                        acc = ppB.tile([128, QKV], F32, tag="acc", bufs=3)
                        for d in range(ND):
                            nc.tensor.matmul(acc[:],
                                             xT_s[:, d, tt * 128:(tt + 1) * 128],
                                             wqkvT_sb[:, d, :],
                                             start=(d == 0), stop=False)
                        nc.tensor.matmul(acc[:], ones_sb[0:1, :], qkvb_sb[:],
                                         start=False, stop=True)

                        # rope on q and k halves (free-dim ops, 5 = 4q + 1k groups)
                        W = QW + HD  # 320
                        tmp = rp.tile([128, W], F32, tag="tmp")
                        qkr = qkrp.tile([128, W], BF16, tag="qkr")
                        acc5 = acc[:, 0:W].rearrange("p (h x) -> p h x", x=HD)
                        tmp5 = tmp[:].rearrange("p (h x) -> p h x", x=HD)
                        nc.vector.tensor_tensor(
                            tmp5[:, :, 0:HD // 2], acc5[:, :, HD // 2:HD],
                            nsin_sb[:, t:t + 1, :].to_broadcast([128, 5, HD // 2]),
                            mybir.AluOpType.mult)
                        nc.vector.tensor_tensor(
                            tmp5[:, :, HD // 2:HD], acc5[:, :, 0:HD // 2],
                            psin_sb[:, t:t + 1, :].to_broadcast([128, 5, HD // 2]),
                            mybir.AluOpType.mult)
                        nc.vector.tensor_tensor(
                            qkr[:], acc[:, 0:W],
                            cos_sb[:, t:t + 1, :].to_broadcast([128, 5, HD]),
                            mybir.AluOpType.mult)
                        nc.vector.tensor_tensor(qkr[:], qkr[:], tmp[:],
                                                mybir.AluOpType.add)
                        qkr_tiles.append(qkr)
                        # v: plain copy (cast to bf16)
                        nc.scalar.copy(v_sb[:, t, 0:HD], acc[:, QW + HD:QKV])

                    # transpose rope'd q/k for the block into [hd, seq] layout
                    for h in range(HPC + 1):
                        tq_ps = ppB.tile([HD, 512], BF16, tag="tq", bufs=2)
                        for tt in range(4):
                            nc.tensor.transpose(
                                tq_ps[:, tt * 128:(tt + 1) * 128],
                                qkr_tiles[tt][:, h * HD:(h + 1) * HD], ident_sb[:])
                        sl = slice(s * SB, (s + 1) * SB)
                        if h < HPC:
                            dst = qP[h // 2][(h % 2) * HD:(h % 2) * HD + HD, sl]
                            if h % 2 == 0:
                                nc.scalar.copy(dst, tq_ps[:])
                            else:
                                nc.vector.tensor_copy(dst, tq_ps[:])
                        else:
                            nc.scalar.copy(kTd[0:HD, sl], tq_ps[:])
                            nc.vector.tensor_copy(kTd[HD:2 * HD, sl], tq_ps[:])

            # load wo^T now: SWDGE queue is idle and the data is only
            # needed by the output projection after the A2A
            nc.gpsimd.dma_start(woT_sb[:], woT_e[:].rearrange(
                "p (o f) -> p o f", o=ND))

            # ---- phase C: attention (flash-style over causal k-tiles) ----
            # Scores for a head PAIR run concurrently on the two 64-row halves
            # of the PE array (tile_position row packing): head 2g on array
            # rows 0-63, head 2g+1 on rows 64-127 (k^T duplicated per half).
            with tc.tile_pool(name="ppC", bufs=2, space="PSUM") as ppC:
                # re-warm the PE clock entering attention (phase B tail is
                # sparse enough to leave it throttled)
                warm3_ps = ppC.tile([128, 512], F32, tag="sc", bufs=3)
                for _ in range(20):
                    nc.tensor.matmul(warm3_ps[:], warm_sb[:, 0:128],
                                     warm_sb[:], start=True, stop=True)
                for g in range(HPC // 2):
                    for s in range(NSB):
                        n_kt = 4 * (s + 1)
                        sl = slice(s * SB, (s + 1) * SB)
                        pvs = [ppC.tile([HD + 1, 512], F32, tag="pv", bufs=2,
                                        name=f"pv{s}_{g}_{z}") for z in range(2)]
                        ptA, ptB = [], []

                        def emit_pv(p, pts, pvz):
                            for half in range(2):
                                i = 2 * p + half
                                nc.tensor.matmul(
                                    pvz[:], v_sb[:, i, :],
                                    pts[p][:, half * 512:(half + 1) * 512],
                                    start=(i == 0), stop=(i == n_kt - 1))

                        for p in range(n_kt // 2):
                            # diagonal k-tiles: q columns < 128*d are fully
                            # masked; skip them in scores and exp (the mask
                            # multiply zeroes stale pt content there, so the
                            # full-width PV accumulation stays correct)
                            offs = []
                            for half in range(2):
                                dd = 2 * p + half - 4 * s
                                offs.append(128 * dd if dd > 0 else 0)
                            scs = []
                            for z in range(2):
                                sc = ppC.tile([128, 1024], F32, tag="sc",
                                              bufs=3, name=f"sc{z}")
                                for half in range(2):
                                    i = 2 * p + half
                                    off = offs[half]
                                    nc.tensor.matmul(
                                        sc[:, half * 512 + off:
                                           (half + 1) * 512],
                                        kTd[z * HD:(z + 1) * HD,
                                            i * 128:(i + 1) * 128],
                                        qP[g][z * HD:(z + 1) * HD,
                                              s * SB + off:(s + 1) * SB],
                                        start=True, stop=True,
                                        tile_position=(z * HD, 0))
                                scs.append(sc)
                            for z, pts in ((0, ptA), (1, ptB)):
                                pt = ptp.tile([128, 1024], BF16, tag="pt")
                                if offs[0] == offs[1] == 0:
                                    nc.scalar.activation(
                                        pt[:], scs[z][:],
                                        mybir.ActivationFunctionType.Exp,
                                        scale=SCALE)
                                else:
                                    for half in range(2):
                                        off = offs[half]
                                        nc.scalar.activation(
                                            pt[:, half * 512 + off:
                                               (half + 1) * 512],
                                            scs[z][:, half * 512 + off:
                                                    (half + 1) * 512],
                                            mybir.ActivationFunctionType.Exp,
                                            scale=SCALE)
                                for half in range(2):
                                    i = 2 * p + half
                                    if i >= 4 * s:
                                        off = offs[half]
                                        # triangular mask on the exact
                                        # diagonal 128-col tile; zero the
                                        # fully-masked cols below it
                                        nc.vector.tensor_tensor(
                                            pt[:, half * 512 + off:
                                               half * 512 + off + 128],
                                            pt[:, half * 512 + off:
                                               half * 512 + off + 128],
                                            mask_sb[:],
                                            mybir.AluOpType.mult)
                                        if off > 0:
                                            nc.vector.memset(
                                                pt[:, half * 512:
                                                   half * 512 + off], 0.0)
                                pts.append(pt)
                            if p % 2 == 1:
                                # grouped PV: 4 same-bank matmuls per head
                                emit_pv(p - 1, ptA, pvs[0])
                                emit_pv(p, ptA, pvs[0])
                                emit_pv(p - 1, ptB, pvs[1])
                                emit_pv(p, ptB, pvs[1])

                        for z in range(2):
                            h = 2 * g + z
                            pv = pvs[z]
                            # epilogue: out_h = pv[0:64] / (S_row + exp(sink_h))
                            srow = epp.tile([1, 512], F32, tag="srow", bufs=2)
                            nc.vector.scalar_tensor_tensor(
                                srow[:], pv[HD:HD + 1, :], es_sb[0:1, h:h + 1],
                                ones512[:], mybir.AluOpType.add,
                                mybir.AluOpType.mult)
                            rrow = epp.tile([1, 512], F32, tag="rrow", bufs=2)
                            nc.vector.reciprocal_approx_fast(rrow[:], srow[:])
                            rbc = epp.tile([HD, 512], F32, tag="rbc", bufs=3)
                            nc.gpsimd.partition_broadcast(rbc[:], rrow[0:1, :])
                            nc.vector.tensor_tensor(
                                oT[h][:, sl], pv[0:HD, :], rbc[:],
                                mybir.AluOpType.mult)
                            # stream this (head, block) slice into its A2A shards
                            nc.sync.dma_start(
                                a2a_in[g][:].rearrange(
                                    "(j hh p) n -> hh p j n",
                                    j=NCORES, hh=2)[z][:, 2 * s:2 * s + 2],
                                oT[h][:, sl].rearrange(
                                    "p (j n) -> p j n", j=2))

                    # redistribute this head pair: heads -> sequence slices.
                    # The g=0 A2A overlaps with the g=1 attention pass, the
                    # g=1 A2A with the first half of the output projection.
                    nc.gpsimd.collective_compute(
                        "AllToAll", mybir.AluOpType.bypass,
                        replica_groups=[list(range(NCORES))],
                        ins=[a2a_in[g].opt()], outs=[a2a_out[g].opt()])
                    ago = a2a_out[g][:].rearrange("(o p) n -> p o n", p=128)
                    nc.sync.dma_start(ag_sb[g][:, 0:4, :], ago[:, 0:4, :])
                    nc.sync.dma_start(ag_sb[g][:, 4:8, :], ago[:, 4:8, :])

                # re-warm the PE while the ag0 readback lands (inside ppC so
                # the warm target reuses an sc bank with no WAR wait; a warm
                # in ppD would stall on phase C's last psum readers)
                warm_t = ppC.tile([128, 512], F32, tag="sc", bufs=3,
                                  name="warmT")
                for _ in range(10):
                    nc.tensor.matmul(warm_t[:], warm_sb[:, 0:128],
                                     warm_sb[:], start=True, stop=True)

            # ---- output projection for my sequence slice ----
            # All 8 (m,n) psum tiles stay open: the g=0 half-contraction
            # (ag0 x even woT blocks) runs while the g=1 A2A is in flight,
            # then the g=1 half + bias close each accumulation.
            with tc.tile_pool(name="ppD", bufs=8, space="PSUM") as ppD:
                fps = [ppD.tile([128, 512], F32, tag="fp", bufs=8,
                                name=f"fp{t}") for t in range(8)]
                for m in range(MYS // 128):
                    for n in range(DIM // 512):
                        fp = fps[m * 4 + n]
                        for c in range(NCORES):
                            nc.tensor.matmul(
                                fp[:], ag_sb[0][:, c, m * 128:(m + 1) * 128],
                                woT_sb[:, 2 * c, n * 512:(n + 1) * 512],
                                start=(c == 0), stop=False)
                for m in range(MYS // 128):
                    for n in range(DIM // 512):
                        fp = fps[m * 4 + n]
                        for c in range(NCORES):
                            nc.tensor.matmul(
                                fp[:], ag_sb[1][:, c, m * 128:(m + 1) * 128],
                                woT_sb[:, 2 * c + 1, n * 512:(n + 1) * 512],
                                start=False, stop=False)
                        nc.tensor.matmul(fp[:], ones_sb[0:1, :],
                                         wob_sb[0:1, n * 512:(n + 1) * 512],
                                         start=False, stop=True)
                        fo = fnp.tile([128, 512], F32, tag="fo")
                        if (m * 4 + n) % 2 == 0:
                            nc.scalar.copy(fo[:], fp[:])
                        else:
                            nc.vector.tensor_copy(fo[:], fp[:])
                        nc.sync.dma_start(
                            out_e[m * 128:(m + 1) * 128,
                                  n * 512:(n + 1) * 512], fo[:])

    nc.compile()
    return nc


def _host_prep(x, rope_cache, wq_w, wq_b, wk_w, wk_b, wv_w, wv_b,
               wo_w, wo_b, sinks):
    """Build the per-core input maps (sharding + layout prep)."""
    x2 = np.asarray(x, np.float32).reshape(S, DIM).astype(ml_dtypes.bfloat16)
    # x^T packed partition-major by dim tile, one param per 512-row seq block
    xTp = x2.T.reshape(ND, 128, S).transpose(1, 0, 2)  # [128, ND, S]
    xT_blocks = [np.ascontiguousarray(
        xTp[:, :, s * SB:(s + 1) * SB]).reshape(128, ND * SB)
        for s in range(NSB)]
    def _pm(a):
        # [S, F] -> [128, (S//128) * F] partition-major packing
        f = a.shape[1]
        return np.ascontiguousarray(
            a.reshape(S // 128, 128, f).transpose(1, 0, 2).reshape(
                128, (S // 128) * f))

    cos = np.asarray(rope_cache[:, :HD // 2], np.float32)
    sin = np.asarray(rope_cache[:, HD // 2:], np.float32)
    cosd = _pm(np.concatenate([cos, cos], axis=1)).astype(np.float16)
    nsin = _pm(-sin).astype(np.float16)
    psin = _pm(sin).astype(np.float16)
    # triangular causal mask for the exact diagonal 128x128 k-q tile
    masks = np.triu(np.ones((128, 128), np.float32)).astype(ml_dtypes.bfloat16)
    masks = np.ascontiguousarray(masks)
    ident = np.eye(128, dtype=ml_dtypes.bfloat16)
    woT = np.asarray(wo_w, np.float32).T.astype(ml_dtypes.bfloat16)
    woT = np.ascontiguousarray(
        woT.reshape(ND, 128, DIM).transpose(1, 0, 2).reshape(128, ND * DIM))
    wob = np.asarray(wo_b, np.float32).astype(
        ml_dtypes.bfloat16).reshape(1, DIM)

    in_maps = []
    for c in range(NCORES):
        qsl = slice(c * QW, (c + 1) * QW)
        ksl = slice(c * HD, (c + 1) * HD)
        wqkvT = np.concatenate([
            np.asarray(wq_w, np.float32)[qsl].T,
            np.asarray(wk_w, np.float32)[ksl].T,
            np.asarray(wv_w, np.float32)[ksl].T],
            axis=1).astype(ml_dtypes.bfloat16)
        wqkvT = np.ascontiguousarray(
            wqkvT.reshape(ND, 128, QKV).transpose(1, 0, 2).reshape(
                128, ND * QKV))
        qkvb = np.ascontiguousarray(np.concatenate([
            np.asarray(wq_b, np.float32)[qsl],
            np.asarray(wk_b, np.float32)[ksl],
            np.asarray(wv_b, np.float32)[ksl]]).astype(
                ml_dtypes.bfloat16)).reshape(1, QKV)
        sinks4 = np.ascontiguousarray(
            np.asarray(sinks, np.float32)[c * HPC:(c + 1) * HPC]).reshape(1, HPC)
        im = {
            "wqkvT": wqkvT, "qkvb": qkvb, "cosd": cosd,
            "nsin": nsin, "psin": psin, "masks": masks, "ident": ident,
            "woT": woT, "wob": wob, "sinks4": sinks4,
        }
        for s in range(NSB):
            im[f"xT{s}"] = xT_blocks[s]
        in_maps.append(im)
    return in_maps


def kernel(**inputs):
    global last_exec_time_ns
    if "nc" not in _cache:
        _cache["nc"] = _build()
    nc = _cache["nc"]
    in_maps = _host_prep(**inputs)
    trace = bool(int(os.environ.get("BASS_KERNEL_TRACE", "0")))
    if trace:
        try:
            _install_ntff_shim()
        except Exception:
            trace = False
    tc_env = os.environ.get("BASS_KERNEL_TRACE_CORES")
    kw = {}
    if trace and tc_env:
        kw["trace_cores"] = [int(c) for c in tc_env.split(",")]
    res = run_bass_kernel_spmd(nc, in_maps, core_ids=list(range(NCORES)),
                               trace=trace, **kw)
    last_exec_time_ns = res.exec_time_ns
    out = np.concatenate([res.results[c]["out"] for c in range(NCORES)],
                         axis=0)
    return out.reshape(B, S, NH * HD)



# revision 34
# speedup vs baseline: 1.7712x; 1.7712x over previous
"""Trainium2 Bass kernel for GQA attention with RoPE, causal mask, and
attention sinks (nn_Attention_65094524338392).

Sharding: tensor-parallel by heads across 8 NeuronCores. Core c owns query
heads 4c..4c+3 and kv-head c (NREP=4, so kv-head groups stay aligned with
their query heads). Each core computes QKV projections over the full
sequence for its heads, flash-style causal attention, then an AllToAll
redistributes attention outputs from head-sharding to sequence-sharding so
each core computes the output projection for its 256-row sequence slice.

Math note: the sink scaling folds into the softmax normalizer:
    out = (sum_k exp(s_k) v_k) * sigmoid(lse - sink) / sum_k exp(s_k)
        = (sum_k exp(s_k) v_k) / (sum_k exp(s_k) + exp(sink))
so no logs/sigmoids are needed on device, and because |s| <= ~40 no
max-subtraction is needed for exp stability in fp32 accumulation.
"""

import os
import sys

sys.path.insert(0, "/opt/trn_rl_repo")

import ml_dtypes
import numpy as np

import concourse.bass as bass
import concourse.mybir as mybir
import concourse.tile as tile
from concourse import bacc
from concourse.bass_utils import run_bass_kernel_spmd

# Problem shapes
B, S, DIM = 1, 2048, 2048
NH, NKV, HD = 32, 8, 64
NREP = NH // NKV
SCALE = 1.0 / float(np.sqrt(HD))
NCORES = 8
HPC = NH // NCORES            # query heads per core (4)
QKV = HPC * HD + 2 * HD       # fused qkv output dim per core (384)
QW = HPC * HD                 # query width per core (256)
SB = 512                      # seq block (attention q-block)
NSB = S // SB                 # 4
NT = S // 128                 # 16 seq tiles
ND = DIM // 128               # 16 contraction tiles
MYS = S // NCORES             # output rows per core (256)

F32 = mybir.dt.float32
BF16 = mybir.dt.bfloat16

_cache = {}

last_exec_time_ns = None


def _install_ntff_shim():
    """Register the NTFF profile hook so trace=True yields exec_time_ns."""
    import types
    if "antenv.axon_hooks" in sys.modules:
        return
    import antenv
    mod = types.ModuleType("antenv.axon_hooks")
    mod._hook = None
    mod.set_axon_ntff_profile_hook = lambda h: setattr(mod, "_hook", h)
    mod.get_axon_ntff_profile_hook = lambda: mod._hook
    sys.modules["antenv.axon_hooks"] = mod
    antenv.axon_hooks = mod
    from trn_agent_boot.trn_boot import _ntff_profile_via_ctypes
    hook = _ntff_profile_via_ctypes("/opt/axon/libaxon_pjrt.so")
    if hook is not None:
        mod._hook = hook


def _build():
    nc = bacc.Bacc("TRN2", target_bir_lowering=False, debug=False,
                   num_devices=NCORES)

    # Input staging happens in declaration order; QKV needs wqkvT + xT0
    # first, so they stage right after the tiny params. cos/sin/masks are
    # pulled on the gpsimd queue so they don't block the sync queue's xT
    # pulls.
    F16 = mybir.dt.float16
    ident_e = nc.declare_dram_parameter("ident", [128, 128], BF16, isOutput=False)
    qkvb_e = nc.declare_dram_parameter("qkvb", [1, QKV], BF16, isOutput=False)
    mask_e = nc.declare_dram_parameter("masks", [128, 128], BF16, isOutput=False)
    wqkvT_e = nc.declare_dram_parameter("wqkvT", [128, ND * QKV], BF16, isOutput=False)
    # x^T is transposed host-side and staged as one param per 512-row seq
    # block so phase B can start as soon as block 0 lands
    xT_es = [nc.declare_dram_parameter(f"xT{s}", [128, ND * SB], BF16,
                                       isOutput=False) for s in range(2)]
    cosd_e = nc.declare_dram_parameter("cosd", [128, NT * HD], F16, isOutput=False)
    nsin_e = nc.declare_dram_parameter("nsin", [128, NT * HD // 2], F16, isOutput=False)
    psin_e = nc.declare_dram_parameter("psin", [128, NT * HD // 2], F16, isOutput=False)
    sinks_e = nc.declare_dram_parameter("sinks4", [1, HPC], F32, isOutput=False)
    xT_es += [nc.declare_dram_parameter(f"xT{s}", [128, ND * SB], BF16,
                                        isOutput=False) for s in range(2, 3)]
    wob_e = nc.declare_dram_parameter("wob", [1, DIM], BF16, isOutput=False)
    xT_es += [nc.declare_dram_parameter(f"xT{s}", [128, ND * SB], BF16,
                                        isOutput=False) for s in range(3, NSB)]
    woT_e = nc.declare_dram_parameter("woT", [128, ND * DIM], BF16, isOutput=False)
    out_e = nc.declare_dram_parameter("out", [MYS, DIM], F32, isOutput=True)

    with tile.TileContext(nc) as tc:
        with tc.tile_pool(name="const", bufs=1) as cp, \
             tc.tile_pool(name="xT", bufs=2) as xtp, \
             tc.tile_pool(name="rope", bufs=3) as rp, \
             tc.tile_pool(name="qkr", bufs=5) as qkrp, \
             tc.tile_pool(name="pt", bufs=5) as ptp, \
             tc.tile_pool(name="ep", bufs=2) as epp, \
             tc.tile_pool(name="fin", bufs=3) as fnp, \
             tc.tile_pool(name="dram", bufs=1, space="DRAM") as dp:

            # ---- constants ---- sync queue carries only what QKV needs
            # (ident/qkvb/wqkvT, then the xT pulls in phase B); everything
            # else rides the gpsimd queue in parallel.
            ident_sb = cp.tile([128, 128], BF16)
            nc.sync.dma_start(ident_sb[:], ident_e[:])
            qkvb_sb = cp.tile([1, QKV], BF16)
            nc.sync.dma_start(qkvb_sb[:], qkvb_e[:])
            wqkvT_sb = cp.tile([128, ND, QKV], BF16)
            nc.sync.dma_start(wqkvT_sb[:], wqkvT_e[:].rearrange(
                "p (o f) -> p o f", o=ND))
            # gpsimd memsets first (instant, and the warm-up matmuls gate
            # on warm_sb), then the gpsimd const pulls
            ones_sb = cp.tile([1, 128], BF16)
            nc.gpsimd.memset(ones_sb[:], 1.0)
            ones512 = cp.tile([1, 512], BF16)
            nc.gpsimd.memset(ones512[:], 1.0)
            warm_sb = cp.tile([128, 512], BF16)
            nc.gpsimd.memset(warm_sb[:], 0.0)
            v_sb = cp.tile([128, NT, HD + 1], BF16)
            nc.gpsimd.memset(v_sb[:, :, HD:HD + 1], 1.0)
            # zero the pt pool buffers once: diagonal-trimmed exp leaves
            # the masked column range unwritten, and the mask multiply
            # must see finite values there (not uninitialized SBUF)
            for pi in range(5):
                ptz = ptp.tile([128, 1024], BF16, tag="pt", name=f"ptz{pi}")
                nc.gpsimd.memset(ptz[:, :], 0.0)
            mask_sb = cp.tile([128, 128], BF16)
            nc.gpsimd.dma_start(mask_sb[:], mask_e[:])
            cos_sb = cp.tile([128, NT, HD], F16)
            nc.gpsimd.dma_start(cos_sb[:], cosd_e[:].rearrange(
                "p (o f) -> p o f", o=NT))
            nsin_sb = cp.tile([128, NT, HD // 2], F16)
            nc.gpsimd.dma_start(nsin_sb[:], nsin_e[:].rearrange(
                "p (o f) -> p o f", o=NT))
            psin_sb = cp.tile([128, NT, HD // 2], F16)
            nc.gpsimd.dma_start(psin_sb[:], psin_e[:].rearrange(
                "p (o f) -> p o f", o=NT))
            sinks_sb = cp.tile([1, HPC], F32)
            nc.gpsimd.dma_start(sinks_sb[:], sinks_e[:])
            es_sb = cp.tile([1, HPC], F32)
            nc.scalar.activation(es_sb[:], sinks_sb[:],
                                 mybir.ActivationFunctionType.Exp)
            wob_sb = cp.tile([1, DIM], BF16)
            nc.gpsimd.dma_start(wob_sb[:], wob_e[:])
            woT_sb = cp.tile([128, ND, DIM], BF16)

            # PE warm-up: a gapless burst of dummy matmuls during the
            # initial input DMAs releases the HAM clock throttle (1.2 ->
            # 2.4 GHz) before real work begins. No data dependencies.
            with tc.tile_pool(name="warm", bufs=1, space="PSUM") as wpp:
                warm_ps = wpp.tile([128, 512], F32, tag="warm")
                for _ in range(30):
                    nc.tensor.matmul(warm_ps[:], warm_sb[:, 0:128],
                                     warm_sb[:], start=True, stop=True)

            # persistent activations: q head-pairs packed in partition halves,
            # k duplicated into both halves (for tile_position row packing)
            qP = [cp.tile([128, S], BF16, name=f"qP{g}") for g in range(HPC // 2)]
            kTd = cp.tile([128, S], BF16)
            oT = [cp.tile([HD, S], BF16, name=f"oT{h}") for h in range(HPC)]

            # one A2A per head pair: the first fires after the g=0 pass and
            # overlaps with the g=1 attention pass
            a2a_in = [dp.tile([S // 2, MYS], BF16, name=f"a2ai{g}")
                      for g in range(HPC // 2)]
            a2a_out = [dp.tile([S // 2, MYS], BF16, name=f"a2ao{g}")
                       for g in range(HPC // 2)]
            # gathered attention outputs: ag_sb[g] block c holds global head
            # rows [c*256 + 128*g, c*256 + 128*(g+1)) = woT block 2c+g
            ag_sb = [cp.tile([128, NCORES, MYS], BF16, name=f"ag{g}")
                     for g in range(HPC // 2)]

            # ---- phase B: QKV projections + rope + transposes ----
            with tc.tile_pool(name="ppB", bufs=2, space="PSUM") as ppB:
                for s in range(NSB):
                    # x^T was transposed host-side; two bulk loads per block
                    xT_s = xtp.tile([128, ND, SB], BF16, tag="xT")
                    xsrc = xT_es[s][:].rearrange("p (d f) -> p d f", d=ND)
                    nc.sync.dma_start(xT_s[:, 0:ND // 2, :],
                                      xsrc[:, 0:ND // 2, :])
                    nc.sync.dma_start(xT_s[:, ND // 2:, :],
                                      xsrc[:, ND // 2:, :])
                    if s == 0:
                        # dependency-gated warm-up: fires as soon as the first
                        # x^T tile lands, so QKV starts at the warm PE clock
                        warm2_ps = ppB.tile([128, 512], F32, tag="warmB",
                                            bufs=1)
                        for _ in range(20):
                            nc.tensor.matmul(warm2_ps[:], xT_s[:, 0, 0:128],
                                             warm_sb[:], start=True, stop=True)
                    qkr_tiles = []
                    for tt in range(4):
                        t = 4 * s + tt

                        # fused qkv projection for this seq tile
                        acc = ppB.tile([128, QKV], F32, tag="acc", bufs=3)
                        for d in range(ND):
                            nc.tensor.matmul(acc[:],
                                             xT_s[:, d, tt * 128:(tt + 1) * 128],
                                             wqkvT_sb[:, d, :],
                                             start=(d == 0), stop=False)
                        nc.tensor.matmul(acc[:], ones_sb[0:1, :], qkvb_sb[:],
                                         start=False, stop=True)

                        # rope on q and k halves (free-dim ops, 5 = 4q + 1k groups)
                        W = QW + HD  # 320
                        tmp = rp.tile([128, W], F32, tag="tmp")
                        qkr = qkrp.tile([128, W], BF16, tag="qkr")
                        acc5 = acc[:, 0:W].rearrange("p (h x) -> p h x", x=HD)
                        tmp5 = tmp[:].rearrange("p (h x) -> p h x", x=HD)
                        nc.vector.tensor_tensor(
                            tmp5[:, :, 0:HD // 2], acc5[:, :, HD // 2:HD],
                            nsin_sb[:, t:t + 1, :].to_broadcast([128, 5, HD // 2]),
                            mybir.AluOpType.mult)
                        nc.vector.tensor_tensor(
                            tmp5[:, :, HD // 2:HD], acc5[:, :, 0:HD // 2],
                            psin_sb[:, t:t + 1, :].to_broadcast([128, 5, HD // 2]),
                            mybir.AluOpType.mult)
                        nc.vector.tensor_tensor(
                            qkr[:], acc[:, 0:W],
                            cos_sb[:, t:t + 1, :].to_broadcast([128, 5, HD]),
                            mybir.AluOpType.mult)
                        nc.vector.tensor_tensor(qkr[:], qkr[:], tmp[:],
                                                mybir.AluOpType.add)
                        qkr_tiles.append(qkr)
                        # v: plain copy (cast to bf16)
                        nc.scalar.copy(v_sb[:, t, 0:HD], acc[:, QW + HD:QKV])

                    # transpose rope'd q/k for the block into [hd, seq] layout
                    for h in range(HPC + 1):
                        tq_ps = ppB.tile([HD, 512], BF16, tag="tq", bufs=2)
                        for tt in range(4):
                            nc.tensor.transpose(
                                tq_ps[:, tt * 128:(tt + 1) * 128],
                                qkr_tiles[tt][:, h * HD:(h + 1) * HD], ident_sb[:])
                        sl = slice(s * SB, (s + 1) * SB)
                        if h < HPC:
                            dst = qP[h // 2][(h % 2) * HD:(h % 2) * HD + HD, sl]
                            if h % 2 == 0:
                                nc.scalar.copy(dst, tq_ps[:])
                            else:
                                nc.vector.tensor_copy(dst, tq_ps[:])
                        else:
                            nc.scalar.copy(kTd[0:HD, sl], tq_ps[:])
                            nc.vector.tensor_copy(kTd[HD:2 * HD, sl], tq_ps[:])

            # load wo^T now: SWDGE queue is idle and the data is only
            # needed by the output projection after the A2A
            nc.gpsimd.dma_start(woT_sb[:], woT_e[:].rearrange(
                "p (o f) -> p o f", o=ND))

            # ---- phase C: attention (flash-style over causal k-tiles) ----
            # Scores for a head PAIR run concurrently on the two 64-row halves
            # of the PE array (tile_position row packing): head 2g on array
            # rows 0-63, head 2g+1 on rows 64-127 (k^T duplicated per half).
            with tc.tile_pool(name="ppC", bufs=2, space="PSUM") as ppC:
                # re-warm the PE clock entering attention (phase B tail is
                # sparse enough to leave it throttled)
                warm3_ps = ppC.tile([128, 512], F32, tag="sc", bufs=3)
                for _ in range(20):
                    nc.tensor.matmul(warm3_ps[:], warm_sb[:, 0:128],
                                     warm_sb[:], start=True, stop=True)
                # Emission schedule: pull (g1,s0) and (g1,s1) early — their
                # inputs are ready while xT1/xT2 are still staging, filling
                # the startup PE gaps — but keep (g1,s2)+(g1,s3) after the
                # g=0 A2A so it still has a ~28us hiding window.
                SCHED = [(0, 0), (1, 0), (0, 1), (1, 1), (0, 2), (0, 3),
                         "a2a0", (1, 2), (1, 3), "a2a1"]
                for step in SCHED:
                    if isinstance(step, str):
                        g = int(step[3])
                        nc.gpsimd.collective_compute(
                            "AllToAll", mybir.AluOpType.bypass,
                            replica_groups=[list(range(NCORES))],
                            ins=[a2a_in[g].opt()], outs=[a2a_out[g].opt()])
                        ago = a2a_out[g][:].rearrange(
                            "(o p) n -> p o n", p=128)
                        nc.sync.dma_start(ag_sb[g][:, 0:4, :], ago[:, 0:4, :])
                        nc.sync.dma_start(ag_sb[g][:, 4:8, :], ago[:, 4:8, :])
                        continue
                    g, s = step
                    if True:
                        n_kt = 4 * (s + 1)
                        sl = slice(s * SB, (s + 1) * SB)
                        pvs = [ppC.tile([HD + 1, 512], F32, tag="pv", bufs=2,
                                        name=f"pv{s}_{g}_{z}") for z in range(2)]
                        ptA, ptB = [], []

                        def emit_pv(p, pts, pvz):
                            for half in range(2):
                                i = 2 * p + half
                                nc.tensor.matmul(
                                    pvz[:], v_sb[:, i, :],
                                    pts[p][:, half * 512:(half + 1) * 512],
                                    start=(i == 0), stop=(i == n_kt - 1))

                        for p in range(n_kt // 2):
                            # diagonal k-tiles: q columns < 128*d are fully
                            # masked; skip them in scores and exp (the mask
                            # multiply zeroes stale pt content there, so the
                            # full-width PV accumulation stays correct)
                            offs = []
                            for half in range(2):
                                dd = 2 * p + half - 4 * s
                                offs.append(128 * dd if dd > 0 else 0)
                            scs = []
                            for z in range(2):
                                sc = ppC.tile([128, 1024], F32, tag="sc",
                                              bufs=3, name=f"sc{z}")
                                for half in range(2):
                                    i = 2 * p + half
                                    off = offs[half]
                                    nc.tensor.matmul(
                                        sc[:, half * 512 + off:
                                           (half + 1) * 512],
                                        kTd[z * HD:(z + 1) * HD,
                                            i * 128:(i + 1) * 128],
                                        qP[g][z * HD:(z + 1) * HD,
                                              s * SB + off:(s + 1) * SB],
                                        start=True, stop=True,
                                        tile_position=(z * HD, 0))
                                scs.append(sc)
                            for z, pts in ((0, ptA), (1, ptB)):
                                pt = ptp.tile([128, 1024], BF16, tag="pt")
                                if offs[0] == offs[1] == 0:
                                    nc.scalar.activation(
                                        pt[:], scs[z][:],
                                        mybir.ActivationFunctionType.Exp,
                                        scale=SCALE)
                                else:
                                    for half in range(2):
                                        off = offs[half]
                                        nc.scalar.activation(
                                            pt[:, half * 512 + off:
                                               (half + 1) * 512],
                                            scs[z][:, half * 512 + off:
                                                    (half + 1) * 512],
                                            mybir.ActivationFunctionType.Exp,
                                            scale=SCALE)
                                for half in range(2):
                                    i = 2 * p + half
                                    if i >= 4 * s:
                                        off = offs[half]
                                        # triangular mask on the exact
                                        # diagonal 128-col tile; zero the
                                        # fully-masked cols below it
                                        nc.vector.tensor_tensor(
                                            pt[:, half * 512 + off:
                                               half * 512 + off + 128],
                                            pt[:, half * 512 + off:
                                               half * 512 + off + 128],
                                            mask_sb[:],
                                            mybir.AluOpType.mult)
                                        if off > 0:
                                            nc.vector.memset(
                                                pt[:, half * 512:
                                                   half * 512 + off], 0.0)
                                pts.append(pt)
                            if p % 2 == 1:
                                # grouped PV: 4 same-bank matmuls per head
                                emit_pv(p - 1, ptA, pvs[0])
                                emit_pv(p, ptA, pvs[0])
                                emit_pv(p - 1, ptB, pvs[1])
                                emit_pv(p, ptB, pvs[1])

                        for z in range(2):
                            h = 2 * g + z
                            pv = pvs[z]
                            # epilogue: out_h = pv[0:64] / (S_row + exp(sink_h))
                            srow = epp.tile([1, 512], F32, tag="srow", bufs=2)
                            nc.vector.scalar_tensor_tensor(
                                srow[:], pv[HD:HD + 1, :], es_sb[0:1, h:h + 1],
                                ones512[:], mybir.AluOpType.add,
                                mybir.AluOpType.mult)
                            rrow = epp.tile([1, 512], F32, tag="rrow", bufs=2)
                            nc.vector.reciprocal_approx_fast(rrow[:], srow[:])
                            rbc = epp.tile([HD, 512], F32, tag="rbc", bufs=3)
                            nc.gpsimd.partition_broadcast(rbc[:], rrow[0:1, :])
                            nc.vector.tensor_tensor(
                                oT[h][:, sl], pv[0:HD, :], rbc[:],
                                mybir.AluOpType.mult)
                            # stream this (head, block) slice into its A2A shards
                            nc.sync.dma_start(
                                a2a_in[g][:].rearrange(
                                    "(j hh p) n -> hh p j n",
                                    j=NCORES, hh=2)[z][:, 2 * s:2 * s + 2],
                                oT[h][:, sl].rearrange(
                                    "p (j n) -> p j n", j=2))

                # re-warm the PE while the ag0 readback lands (inside ppC so
                # the warm target reuses an sc bank with no WAR wait; a warm
                # in ppD would stall on phase C's last psum readers)
                warm_t = ppC.tile([128, 512], F32, tag="sc", bufs=3,
                                  name="warmT")
                for _ in range(10):
                    nc.tensor.matmul(warm_t[:], warm_sb[:, 0:128],
                                     warm_sb[:], start=True, stop=True)

            # ---- output projection for my sequence slice ----
            # All 8 (m,n) psum tiles stay open: the g=0 half-contraction
            # (ag0 x even woT blocks) runs while the g=1 A2A is in flight,
            # then the g=1 half + bias close each accumulation.
            with tc.tile_pool(name="ppD", bufs=8, space="PSUM") as ppD:
                fps = [ppD.tile([128, 512], F32, tag="fp", bufs=8,
                                name=f"fp{t}") for t in range(8)]
                for m in range(MYS // 128):
                    for n in range(DIM // 512):
                        fp = fps[m * 4 + n]
                        for c in range(NCORES):
                            nc.tensor.matmul(
                                fp[:], ag_sb[0][:, c, m * 128:(m + 1) * 128],
                                woT_sb[:, 2 * c, n * 512:(n + 1) * 512],
                                start=(c == 0), stop=False)
                for m in range(MYS // 128):
                    for n in range(DIM // 512):
                        fp = fps[m * 4 + n]
                        for c in range(NCORES):
                            nc.tensor.matmul(
                                fp[:], ag_sb[1][:, c, m * 128:(m + 1) * 128],
                                woT_sb[:, 2 * c + 1, n * 512:(n + 1) * 512],
                                start=False, stop=False)
                        nc.tensor.matmul(fp[:], ones_sb[0:1, :],
                                         wob_sb[0:1, n * 512:(n + 1) * 512],
                                         start=False, stop=True)
                        fo = fnp.tile([128, 512], F32, tag="fo")
                        if (m * 4 + n) % 2 == 0:
                            nc.scalar.copy(fo[:], fp[:])
                        else:
                            nc.vector.tensor_copy(fo[:], fp[:])
                        nc.sync.dma_start(
                            out_e[m * 128:(m + 1) * 128,
                                  n * 512:(n + 1) * 512], fo[:])

    nc.compile()
    return nc


def _host_prep(x, rope_cache, wq_w, wq_b, wk_w, wk_b, wv_w, wv_b,
               wo_w, wo_b, sinks):
    """Build the per-core input maps (sharding + layout prep)."""
    x2 = np.asarray(x, np.float32).reshape(S, DIM).astype(ml_dtypes.bfloat16)
    # x^T packed partition-major by dim tile, one param per 512-row seq block
    xTp = x2.T.reshape(ND, 128, S).transpose(1, 0, 2)  # [128, ND, S]
    xT_blocks = [np.ascontiguousarray(
        xTp[:, :, s * SB:(s + 1) * SB]).reshape(128, ND * SB)
        for s in range(NSB)]
    def _pm(a):
        # [S, F] -> [128, (S//128) * F] partition-major packing
        f = a.shape[1]
        return np.ascontiguousarray(
            a.reshape(S // 128, 128, f).transpose(1, 0, 2).reshape(
                128, (S // 128) * f))

    cos = np.asarray(rope_cache[:, :HD // 2], np.float32)
    sin = np.asarray(rope_cache[:, HD // 2:], np.float32)
    cosd = _pm(np.concatenate([cos, cos], axis=1)).astype(np.float16)
    nsin = _pm(-sin).astype(np.float16)
    psin = _pm(sin).astype(np.float16)
    # triangular causal mask for the exact diagonal 128x128 k-q tile
    masks = np.triu(np.ones((128, 128), np.float32)).astype(ml_dtypes.bfloat16)
    masks = np.ascontiguousarray(masks)
    ident = np.eye(128, dtype=ml_dtypes.bfloat16)
    woT = np.asarray(wo_w, np.float32).T.astype(ml_dtypes.bfloat16)
    woT = np.ascontiguousarray(
        woT.reshape(ND, 128, DIM).transpose(1, 0, 2).reshape(128, ND * DIM))
    wob = np.asarray(wo_b, np.float32).astype(
        ml_dtypes.bfloat16).reshape(1, DIM)

    in_maps = []
    for c in range(NCORES):
        qsl = slice(c * QW, (c + 1) * QW)
        ksl = slice(c * HD, (c + 1) * HD)
        wqkvT = np.concatenate([
            np.asarray(wq_w, np.float32)[qsl].T,
            np.asarray(wk_w, np.float32)[ksl].T,
            np.asarray(wv_w, np.float32)[ksl].T],
            axis=1).astype(ml_dtypes.bfloat16)
        wqkvT = np.ascontiguousarray(
            wqkvT.reshape(ND, 128, QKV).transpose(1, 0, 2).reshape(
                128, ND * QKV))
        qkvb = np.ascontiguousarray(np.concatenate([
            np.asarray(wq_b, np.float32)[qsl],
            np.asarray(wk_b, np.float32)[ksl],
            np.asarray(wv_b, np.float32)[ksl]]).astype(
                ml_dtypes.bfloat16)).reshape(1, QKV)
        sinks4 = np.ascontiguousarray(
            np.asarray(sinks, np.float32)[c * HPC:(c + 1) * HPC]).reshape(1, HPC)
        im = {
            "wqkvT": wqkvT, "qkvb": qkvb, "cosd": cosd,
            "nsin": nsin, "psin": psin, "masks": masks, "ident": ident,
            "woT": woT, "wob": wob, "sinks4": sinks4,
        }
        for s in range(NSB):
            im[f"xT{s}"] = xT_blocks[s]
        in_maps.append(im)
    return in_maps


def kernel(**inputs):
    global last_exec_time_ns
    if "nc" not in _cache:
        _cache["nc"] = _build()
    nc = _cache["nc"]
    in_maps = _host_prep(**inputs)
    trace = bool(int(os.environ.get("BASS_KERNEL_TRACE", "0")))
    if trace:
        try:
            _install_ntff_shim()
        except Exception:
            trace = False
    tc_env = os.environ.get("BASS_KERNEL_TRACE_CORES")
    kw = {}
    if trace and tc_env:
        kw["trace_cores"] = [int(c) for c in tc_env.split(",")]
    res = run_bass_kernel_spmd(nc, in_maps, core_ids=list(range(NCORES)),
                               trace=trace, **kw)
    last_exec_time_ns = res.exec_time_ns
    out = np.concatenate([res.results[c]["out"] for c in range(NCORES)],
                         axis=0)
    return out.reshape(B, S, NH * HD)



# revision 36
# speedup vs baseline: 1.8676x; 1.0544x over previous
"""Trainium2 Bass kernel for GQA attention with RoPE, causal mask, and
attention sinks (nn_Attention_65094524338392).

Sharding: tensor-parallel by heads across 8 NeuronCores. Core c owns query
heads 4c..4c+3 and kv-head c (NREP=4, so kv-head groups stay aligned with
their query heads). Each core computes QKV projections over the full
sequence for its heads, flash-style causal attention, then an AllToAll
redistributes attention outputs from head-sharding to sequence-sharding so
each core computes the output projection for its 256-row sequence slice.

Math note: the sink scaling folds into the softmax normalizer:
    out = (sum_k exp(s_k) v_k) * sigmoid(lse - sink) / sum_k exp(s_k)
        = (sum_k exp(s_k) v_k) / (sum_k exp(s_k) + exp(sink))
so no logs/sigmoids are needed on device, and because |s| <= ~40 no
max-subtraction is needed for exp stability in fp32 accumulation.
"""

import os
import sys

sys.path.insert(0, "/opt/trn_rl_repo")

import ml_dtypes
import numpy as np

import concourse.bass as bass
import concourse.mybir as mybir
import concourse.tile as tile
from concourse import bacc
from concourse.bass_utils import run_bass_kernel_spmd

# Problem shapes
B, S, DIM = 1, 2048, 2048
NH, NKV, HD = 32, 8, 64
NREP = NH // NKV
SCALE = 1.0 / float(np.sqrt(HD))
NCORES = 8
HPC = NH // NCORES            # query heads per core (4)
QKV = HPC * HD + 2 * HD       # fused qkv output dim per core (384)
QW = HPC * HD                 # query width per core (256)
SB = 512                      # seq block (attention q-block)
NSB = S // SB                 # 4
NT = S // 128                 # 16 seq tiles
ND = DIM // 128               # 16 contraction tiles
MYS = S // NCORES             # output rows per core (256)

F32 = mybir.dt.float32
BF16 = mybir.dt.bfloat16

_cache = {}

last_exec_time_ns = None


def _install_ntff_shim():
    """Register the NTFF profile hook so trace=True yields exec_time_ns."""
    import types
    if "antenv.axon_hooks" in sys.modules:
        return
    import antenv
    mod = types.ModuleType("antenv.axon_hooks")
    mod._hook = None
    mod.set_axon_ntff_profile_hook = lambda h: setattr(mod, "_hook", h)
    mod.get_axon_ntff_profile_hook = lambda: mod._hook
    sys.modules["antenv.axon_hooks"] = mod
    antenv.axon_hooks = mod
    from trn_agent_boot.trn_boot import _ntff_profile_via_ctypes
    hook = _ntff_profile_via_ctypes("/opt/axon/libaxon_pjrt.so")
    if hook is not None:
        mod._hook = hook


def _build():
    nc = bacc.Bacc("TRN2", target_bir_lowering=False, debug=False,
                   num_devices=NCORES)

    # Input staging happens in declaration order; QKV needs wqkvT + xT0
    # first, so they stage right after the tiny params. cos/sin/masks are
    # pulled on the gpsimd queue so they don't block the sync queue's xT
    # pulls.
    F16 = mybir.dt.float16
    ident_e = nc.declare_dram_parameter("ident", [128, 128], BF16, isOutput=False)
    qkvb_e = nc.declare_dram_parameter("qkvb", [1, QKV], BF16, isOutput=False)
    mask_e = nc.declare_dram_parameter("masks", [128, 128], BF16, isOutput=False)
    wqkvT_e = nc.declare_dram_parameter("wqkvT", [128, ND * QKV], BF16, isOutput=False)
    # x^T is transposed host-side and staged as one param per 512-row seq
    # block so phase B can start as soon as block 0 lands
    xT_es = [nc.declare_dram_parameter(f"xT{s}", [128, ND * SB], BF16,
                                       isOutput=False) for s in range(2)]
    cosd_e = nc.declare_dram_parameter("cosd", [128, NT * HD], F16, isOutput=False)
    nsin_e = nc.declare_dram_parameter("nsin", [128, NT * HD // 2], F16, isOutput=False)
    psin_e = nc.declare_dram_parameter("psin", [128, NT * HD // 2], F16, isOutput=False)
    sinks_e = nc.declare_dram_parameter("sinks4", [1, HPC], F32, isOutput=False)
    xT_es += [nc.declare_dram_parameter(f"xT{s}", [128, ND * SB], BF16,
                                        isOutput=False) for s in range(2, 3)]
    wob_e = nc.declare_dram_parameter("wob", [1, DIM], BF16, isOutput=False)
    xT_es += [nc.declare_dram_parameter(f"xT{s}", [128, ND * SB], BF16,
                                        isOutput=False) for s in range(3, NSB)]
    woT_e = nc.declare_dram_parameter("woT", [128, ND * DIM], BF16, isOutput=False)
    out_e = nc.declare_dram_parameter("out", [MYS, DIM], F32, isOutput=True)

    with tile.TileContext(nc) as tc:
        with tc.tile_pool(name="const", bufs=1) as cp, \
             tc.tile_pool(name="xT", bufs=2) as xtp, \
             tc.tile_pool(name="rope", bufs=3) as rp, \
             tc.tile_pool(name="qkr", bufs=5) as qkrp, \
             tc.tile_pool(name="pt", bufs=5) as ptp, \
             tc.tile_pool(name="ep", bufs=2) as epp, \
             tc.tile_pool(name="fin", bufs=3) as fnp, \
             tc.tile_pool(name="dram", bufs=1, space="DRAM") as dp:

            # ---- constants ---- sync queue carries only what QKV needs
            # (ident/qkvb/wqkvT, then the xT pulls in phase B); everything
            # else rides the gpsimd queue in parallel.
            ident_sb = cp.tile([128, 128], BF16)
            nc.sync.dma_start(ident_sb[:], ident_e[:])
            qkvb_sb = cp.tile([1, QKV], BF16)
            nc.sync.dma_start(qkvb_sb[:], qkvb_e[:])
            wqkvT_sb = cp.tile([128, ND, QKV], BF16)
            nc.sync.dma_start(wqkvT_sb[:], wqkvT_e[:].rearrange(
                "p (o f) -> p o f", o=ND))
            # gpsimd memsets first (instant, and the warm-up matmuls gate
            # on warm_sb), then the gpsimd const pulls
            ones_sb = cp.tile([1, 128], BF16)
            nc.gpsimd.memset(ones_sb[:], 1.0)
            ones512 = cp.tile([1, 512], BF16)
            nc.gpsimd.memset(ones512[:], 1.0)
            warm_sb = cp.tile([128, 512], BF16)
            nc.gpsimd.memset(warm_sb[:], 0.0)
            v_sb = cp.tile([128, NT, HD + 1], BF16)
            nc.gpsimd.memset(v_sb[:, :, HD:HD + 1], 1.0)
            # zero the pt pool buffers once: diagonal-trimmed exp leaves
            # the masked column range unwritten, and the mask multiply
            # must see finite values there (not uninitialized SBUF)
            for pi in range(5):
                ptz = ptp.tile([128, 1024], BF16, tag="pt", name=f"ptz{pi}")
                nc.gpsimd.memset(ptz[:, :], 0.0)
            mask_sb = cp.tile([128, 128], BF16)
            nc.gpsimd.dma_start(mask_sb[:], mask_e[:])
            cos_sb = cp.tile([128, NT, HD], F16)
            nc.gpsimd.dma_start(cos_sb[:], cosd_e[:].rearrange(
                "p (o f) -> p o f", o=NT))
            nsin_sb = cp.tile([128, NT, HD // 2], F16)
            nc.gpsimd.dma_start(nsin_sb[:], nsin_e[:].rearrange(
                "p (o f) -> p o f", o=NT))
            psin_sb = cp.tile([128, NT, HD // 2], F16)
            nc.gpsimd.dma_start(psin_sb[:], psin_e[:].rearrange(
                "p (o f) -> p o f", o=NT))
            sinks_sb = cp.tile([1, HPC], F32)
            nc.gpsimd.dma_start(sinks_sb[:], sinks_e[:])
            es_sb = cp.tile([1, HPC], F32)
            nc.scalar.activation(es_sb[:], sinks_sb[:],
                                 mybir.ActivationFunctionType.Exp)
            wob_sb = cp.tile([1, DIM], BF16)
            nc.gpsimd.dma_start(wob_sb[:], wob_e[:])
            woT_sb = cp.tile([128, ND, DIM], BF16)

            # PE warm-up: a gapless burst of dummy matmuls during the
            # initial input DMAs releases the HAM clock throttle (1.2 ->
            # 2.4 GHz) before real work begins. No data dependencies.
            with tc.tile_pool(name="warm", bufs=1, space="PSUM") as wpp:
                warm_ps = wpp.tile([128, 512], F32, tag="warm")
                for _ in range(30):
                    nc.tensor.matmul(warm_ps[:], warm_sb[:, 0:128],
                                     warm_sb[:], start=True, stop=True)

            # persistent activations: q head-pairs packed in partition halves,
            # k duplicated into both halves (for tile_position row packing)
            qP = [cp.tile([128, S], BF16, name=f"qP{g}") for g in range(HPC // 2)]
            kTd = cp.tile([128, S], BF16)
            oT = [cp.tile([HD, S], BF16, name=f"oT{h}") for h in range(HPC)]

            # one A2A per head pair: the first fires after the g=0 pass and
            # overlaps with the g=1 attention pass
            a2a_in = [dp.tile([S // 2, MYS], BF16, name=f"a2ai{g}")
                      for g in range(HPC // 2)]
            a2a_out = [dp.tile([S // 2, MYS], BF16, name=f"a2ao{g}")
                       for g in range(HPC // 2)]
            # gathered attention outputs: ag_sb[g] block c holds global head
            # rows [c*256 + 128*g, c*256 + 128*(g+1)) = woT block 2c+g
            ag_sb = [cp.tile([128, NCORES, MYS], BF16, name=f"ag{g}")
                     for g in range(HPC // 2)]

            # ---- phase B: QKV projections + rope + transposes ----
            with tc.tile_pool(name="ppB", bufs=2, space="PSUM") as ppB:
                for s in range(NSB):
                    # x^T was transposed host-side; two bulk loads per block
                    xT_s = xtp.tile([128, ND, SB], BF16, tag="xT")
                    xsrc = xT_es[s][:].rearrange("p (d f) -> p d f", d=ND)
                    nc.sync.dma_start(xT_s[:, 0:ND // 2, :],
                                      xsrc[:, 0:ND // 2, :])
                    nc.sync.dma_start(xT_s[:, ND // 2:, :],
                                      xsrc[:, ND // 2:, :])
                    if s == 0:
                        # dependency-gated warm-up: fires as soon as the first
                        # x^T tile lands, so QKV starts at the warm PE clock
                        warm2_ps = ppB.tile([128, 512], F32, tag="warmB",
                                            bufs=1)
                        for _ in range(20):
                            nc.tensor.matmul(warm2_ps[:], xT_s[:, 0, 0:128],
                                             warm_sb[:], start=True, stop=True)
                    qkr_tiles = []
                    for tt in range(4):
                        t = 4 * s + tt

                        # fused qkv projection for this seq tile
                        acc = ppB.tile([128, QKV], F32, tag="acc", bufs=3)
                        for d in range(ND):
                            nc.tensor.matmul(acc[:],
                                             xT_s[:, d, tt * 128:(tt + 1) * 128],
                                             wqkvT_sb[:, d, :],
                                             start=(d == 0), stop=False)
                        nc.tensor.matmul(acc[:], ones_sb[0:1, :], qkvb_sb[:],
                                         start=False, stop=True)

                        # rope on q and k halves (free-dim ops, 5 = 4q + 1k groups)
                        W = QW + HD  # 320
                        tmp = rp.tile([128, W], F32, tag="tmp")
                        qkr = qkrp.tile([128, W], BF16, tag="qkr")
                        acc5 = acc[:, 0:W].rearrange("p (h x) -> p h x", x=HD)
                        tmp5 = tmp[:].rearrange("p (h x) -> p h x", x=HD)
                        nc.vector.tensor_tensor(
                            tmp5[:, :, 0:HD // 2], acc5[:, :, HD // 2:HD],
                            nsin_sb[:, t:t + 1, :].to_broadcast([128, 5, HD // 2]),
                            mybir.AluOpType.mult)
                        nc.vector.tensor_tensor(
                            tmp5[:, :, HD // 2:HD], acc5[:, :, 0:HD // 2],
                            psin_sb[:, t:t + 1, :].to_broadcast([128, 5, HD // 2]),
                            mybir.AluOpType.mult)
                        nc.vector.tensor_tensor(
                            qkr[:], acc[:, 0:W],
                            cos_sb[:, t:t + 1, :].to_broadcast([128, 5, HD]),
                            mybir.AluOpType.mult)
                        nc.vector.tensor_tensor(qkr[:], qkr[:], tmp[:],
                                                mybir.AluOpType.add)
                        qkr_tiles.append(qkr)
                        # v: plain copy (cast to bf16)
                        nc.scalar.copy(v_sb[:, t, 0:HD], acc[:, QW + HD:QKV])

                    # transpose rope'd q/k for the block into [hd, seq] layout
                    for h in range(HPC + 1):
                        tq_ps = ppB.tile([HD, 512], BF16, tag="tq", bufs=2)
                        for tt in range(4):
                            nc.tensor.transpose(
                                tq_ps[:, tt * 128:(tt + 1) * 128],
                                qkr_tiles[tt][:, h * HD:(h + 1) * HD], ident_sb[:])
                        sl = slice(s * SB, (s + 1) * SB)
                        if h < HPC:
                            dst = qP[h // 2][(h % 2) * HD:(h % 2) * HD + HD, sl]
                            if h % 2 == 0:
                                nc.scalar.copy(dst, tq_ps[:])
                            else:
                                nc.vector.tensor_copy(dst, tq_ps[:])
                        else:
                            nc.scalar.copy(kTd[0:HD, sl], tq_ps[:])
                            nc.vector.tensor_copy(kTd[HD:2 * HD, sl], tq_ps[:])

            # load wo^T now: SWDGE queue is idle and the data is only
            # needed by the output projection after the A2A
            nc.gpsimd.dma_start(woT_sb[:], woT_e[:].rearrange(
                "p (o f) -> p o f", o=ND))

            # ---- phase C: attention (flash-style over causal k-tiles) ----
            # Scores for a head PAIR run concurrently on the two 64-row halves
            # of the PE array (tile_position row packing): head 2g on array
            # rows 0-63, head 2g+1 on rows 64-127 (k^T duplicated per half).
            with tc.tile_pool(name="ppC", bufs=2, space="PSUM") as ppC:
                # re-warm the PE clock entering attention (phase B tail is
                # sparse enough to leave it throttled)
                warm3_ps = ppC.tile([128, 512], F32, tag="sc", bufs=3)
                for _ in range(20):
                    nc.tensor.matmul(warm3_ps[:], warm_sb[:, 0:128],
                                     warm_sb[:], start=True, stop=True)
                for g in range(HPC // 2):
                    for s in range(NSB):
                        n_kt = 4 * (s + 1)
                        sl = slice(s * SB, (s + 1) * SB)
                        pvs = [ppC.tile([HD + 1, 512], F32, tag="pv", bufs=2,
                                        name=f"pv{s}_{g}_{z}") for z in range(2)]
                        ptA, ptB = [], []

                        def emit_pv(p, pts, pvz):
                            for half in range(2):
                                i = 2 * p + half
                                nc.tensor.matmul(
                                    pvz[:], v_sb[:, i, :],
                                    pts[p][:, half * 512:(half + 1) * 512],
                                    start=(i == 0), stop=(i == n_kt - 1))

                        for p in range(n_kt // 2):
                            # diagonal k-tiles: q columns < 128*d are fully
                            # masked; skip them in scores and exp (the mask
                            # multiply zeroes stale pt content there, so the
                            # full-width PV accumulation stays correct)
                            offs = []
                            for half in range(2):
                                dd = 2 * p + half - 4 * s
                                offs.append(128 * dd if dd > 0 else 0)
                            scs = []
                            for z in range(2):
                                sc = ppC.tile([128, 1024], F32, tag="sc",
                                              bufs=3, name=f"sc{z}")
                                for half in range(2):
                                    i = 2 * p + half
                                    off = offs[half]
                                    nc.tensor.matmul(
                                        sc[:, half * 512 + off:
                                           (half + 1) * 512],
                                        kTd[z * HD:(z + 1) * HD,
                                            i * 128:(i + 1) * 128],
                                        qP[g][z * HD:(z + 1) * HD,
                                              s * SB + off:(s + 1) * SB],
                                        start=True, stop=True,
                                        tile_position=(z * HD, 0))
                                scs.append(sc)
                            for z, pts in ((0, ptA), (1, ptB)):
                                pt = ptp.tile([128, 1024], BF16, tag="pt")
                                if offs[0] == offs[1] == 0:
                                    nc.scalar.activation(
                                        pt[:], scs[z][:],
                                        mybir.ActivationFunctionType.Exp,
                                        scale=SCALE)
                                else:
                                    for half in range(2):
                                        off = offs[half]
                                        nc.scalar.activation(
                                            pt[:, half * 512 + off:
                                               (half + 1) * 512],
                                            scs[z][:, half * 512 + off:
                                                    (half + 1) * 512],
                                            mybir.ActivationFunctionType.Exp,
                                            scale=SCALE)
                                for half in range(2):
                                    i = 2 * p + half
                                    if i >= 4 * s:
                                        off = offs[half]
                                        # triangular mask on the exact
                                        # diagonal 128-col tile; zero the
                                        # fully-masked cols below it
                                        nc.vector.tensor_tensor(
                                            pt[:, half * 512 + off:
                                               half * 512 + off + 128],
                                            pt[:, half * 512 + off:
                                               half * 512 + off + 128],
                                            mask_sb[:],
                                            mybir.AluOpType.mult)
                                        if off > 0:
                                            nc.vector.memset(
                                                pt[:, half * 512:
                                                   half * 512 + off], 0.0)
                                pts.append(pt)
                            if p % 2 == 1:
                                # grouped PV: 4 same-bank matmuls per head
                                emit_pv(p - 1, ptA, pvs[0])
                                emit_pv(p, ptA, pvs[0])
                                emit_pv(p - 1, ptB, pvs[1])
                                emit_pv(p, ptB, pvs[1])

                        for z in range(2):
                            h = 2 * g + z
                            pv = pvs[z]
                            # epilogue: out_h = pv[0:64] / (S_row + exp(sink_h))
                            srow = epp.tile([1, 512], F32, tag="srow", bufs=2)
                            nc.vector.scalar_tensor_tensor(
                                srow[:], pv[HD:HD + 1, :], es_sb[0:1, h:h + 1],
                                ones512[:], mybir.AluOpType.add,
                                mybir.AluOpType.mult)
                            rrow = epp.tile([1, 512], F32, tag="rrow", bufs=2)
                            nc.vector.reciprocal_approx_fast(rrow[:], srow[:])
                            rbc = epp.tile([HD, 512], F32, tag="rbc", bufs=3)
                            nc.gpsimd.partition_broadcast(rbc[:], rrow[0:1, :])
                            nc.vector.tensor_tensor(
                                oT[h][:, sl], pv[0:HD, :], rbc[:],
                                mybir.AluOpType.mult)
                            # stream this (head, block) slice into its A2A shards
                            nc.sync.dma_start(
                                a2a_in[g][:].rearrange(
                                    "(j hh p) n -> hh p j n",
                                    j=NCORES, hh=2)[z][:, 2 * s:2 * s + 2],
                                oT[h][:, sl].rearrange(
                                    "p (j n) -> p j n", j=2))

                    # redistribute this head pair: heads -> sequence slices.
                    # The g=0 A2A overlaps with the g=1 attention pass, the
                    # g=1 A2A with the first half of the output projection.
                    nc.gpsimd.collective_compute(
                        "AllToAll", mybir.AluOpType.bypass,
                        replica_groups=[list(range(NCORES))],
                        ins=[a2a_in[g].opt()], outs=[a2a_out[g].opt()])
                    ago = a2a_out[g][:].rearrange("(o p) n -> p o n", p=128)
                    nc.sync.dma_start(ag_sb[g][:, 0:4, :], ago[:, 0:4, :])
                    nc.sync.dma_start(ag_sb[g][:, 4:8, :], ago[:, 4:8, :])

                # re-warm the PE while the ag0 readback lands (inside ppC so
                # the warm target reuses an sc bank with no WAR wait; a warm
                # in ppD would stall on phase C's last psum readers)
                warm_t = ppC.tile([128, 512], F32, tag="sc", bufs=3,
                                  name="warmT")
                for _ in range(10):
                    nc.tensor.matmul(warm_t[:], warm_sb[:, 0:128],
                                     warm_sb[:], start=True, stop=True)

            # ---- output projection for my sequence slice ----
            # All 8 (m,n) psum tiles stay open: the g=0 half-contraction
            # (ag0 x even woT blocks) runs while the g=1 A2A is in flight,
            # then the g=1 half + bias close each accumulation.
            with tc.tile_pool(name="ppD", bufs=8, space="PSUM") as ppD:
                fps = [ppD.tile([128, 512], F32, tag="fp", bufs=8,
                                name=f"fp{t}") for t in range(8)]
                for m in range(MYS // 128):
                    for n in range(DIM // 512):
                        fp = fps[m * 4 + n]
                        for c in range(NCORES):
                            nc.tensor.matmul(
                                fp[:], ag_sb[0][:, c, m * 128:(m + 1) * 128],
                                woT_sb[:, 2 * c, n * 512:(n + 1) * 512],
                                start=(c == 0), stop=False)
                for m in range(MYS // 128):
                    for n in range(DIM // 512):
                        fp = fps[m * 4 + n]
                        for c in range(NCORES):
                            nc.tensor.matmul(
                                fp[:], ag_sb[1][:, c, m * 128:(m + 1) * 128],
                                woT_sb[:, 2 * c + 1, n * 512:(n + 1) * 512],
                                start=False, stop=False)
                        nc.tensor.matmul(fp[:], ones_sb[0:1, :],
                                         wob_sb[0:1, n * 512:(n + 1) * 512],
                                         start=False, stop=True)
                        fo = fnp.tile([128, 512], F32, tag="fo")
                        if (m * 4 + n) % 2 == 0:
                            nc.scalar.copy(fo[:], fp[:])
                        else:
                            nc.vector.tensor_copy(fo[:], fp[:])
                        nc.sync.dma_start(
                            out_e[m * 128:(m + 1) * 128,
                                  n * 512:(n + 1) * 512], fo[:])

    nc.compile()
    return nc


def _host_prep(x, rope_cache, wq_w, wq_b, wk_w, wk_b, wv_w, wv_b,
               wo_w, wo_b, sinks):
    """Build the per-core input maps (sharding + layout prep)."""
    x2 = np.asarray(x, np.float32).reshape(S, DIM).astype(ml_dtypes.bfloat16)
    # x^T packed partition-major by dim tile, one param per 512-row seq block
    xTp = x2.T.reshape(ND, 128, S).transpose(1, 0, 2)  # [128, ND, S]
    xT_blocks = [np.ascontiguousarray(
        xTp[:, :, s * SB:(s + 1) * SB]).reshape(128, ND * SB)
        for s in range(NSB)]
    def _pm(a):
        # [S, F] -> [128, (S//128) * F] partition-major packing
        f = a.shape[1]
        return np.ascontiguousarray(
            a.reshape(S // 128, 128, f).transpose(1, 0, 2).reshape(
                128, (S // 128) * f))

    cos = np.asarray(rope_cache[:, :HD // 2], np.float32)
    sin = np.asarray(rope_cache[:, HD // 2:], np.float32)
    cosd = _pm(np.concatenate([cos, cos], axis=1)).astype(np.float16)
    nsin = _pm(-sin).astype(np.float16)
    psin = _pm(sin).astype(np.float16)
    # triangular causal mask for the exact diagonal 128x128 k-q tile
    masks = np.triu(np.ones((128, 128), np.float32)).astype(ml_dtypes.bfloat16)
    masks = np.ascontiguousarray(masks)
    ident = np.eye(128, dtype=ml_dtypes.bfloat16)
    woT = np.asarray(wo_w, np.float32).T.astype(ml_dtypes.bfloat16)
    woT = np.ascontiguousarray(
        woT.reshape(ND, 128, DIM).transpose(1, 0, 2).reshape(128, ND * DIM))
    wob = np.asarray(wo_b, np.float32).astype(
        ml_dtypes.bfloat16).reshape(1, DIM)

    in_maps = []
    for c in range(NCORES):
        qsl = slice(c * QW, (c + 1) * QW)
        ksl = slice(c * HD, (c + 1) * HD)
        wqkvT = np.concatenate([
            np.asarray(wq_w, np.float32)[qsl].T,
            np.asarray(wk_w, np.float32)[ksl].T,
            np.asarray(wv_w, np.float32)[ksl].T],
            axis=1).astype(ml_dtypes.bfloat16)
        wqkvT = np.ascontiguousarray(
            wqkvT.reshape(ND, 128, QKV).transpose(1, 0, 2).reshape(
                128, ND * QKV))
        qkvb = np.ascontiguousarray(np.concatenate([
            np.asarray(wq_b, np.float32)[qsl],
            np.asarray(wk_b, np.float32)[ksl],
            np.asarray(wv_b, np.float32)[ksl]]).astype(
                ml_dtypes.bfloat16)).reshape(1, QKV)
        sinks4 = np.ascontiguousarray(
            np.asarray(sinks, np.float32)[c * HPC:(c + 1) * HPC]).reshape(1, HPC)
        im = {
            "wqkvT": wqkvT, "qkvb": qkvb, "cosd": cosd,
            "nsin": nsin, "psin": psin, "masks": masks, "ident": ident,
            "woT": woT, "wob": wob, "sinks4": sinks4,
        }
        for s in range(NSB):
            im[f"xT{s}"] = xT_blocks[s]
        in_maps.append(im)
    return in_maps


def kernel(**inputs):
    global last_exec_time_ns
    if "nc" not in _cache:
        _cache["nc"] = _build()
    nc = _cache["nc"]
    in_maps = _host_prep(**inputs)
    trace = bool(int(os.environ.get("BASS_KERNEL_TRACE", "0")))
    if trace:
        try:
            _install_ntff_shim()
        except Exception:
            trace = False
    tc_env = os.environ.get("BASS_KERNEL_TRACE_CORES")
    kw = {}
    if trace and tc_env:
        kw["trace_cores"] = [int(c) for c in tc_env.split(",")]
    res = run_bass_kernel_spmd(nc, in_maps, core_ids=list(range(NCORES)),
                               trace=trace, **kw)
    last_exec_time_ns = res.exec_time_ns
    out = np.concatenate([res.results[c]["out"] for c in range(NCORES)],
                         axis=0)
    return out.reshape(B, S, NH * HD)

